# revision 11
# baseline (speedup 1.0000x reference)
"""Trainium2 Bass kernel for the BayesianLayer problem.

Computes, for full inputs
    x:[128,1024] eps:[128,1024,1024] eps_bias:[128,1024]
    mu,ro:[1024,1024] mu_bias,ro_bias:[1,1024]  (all fp32):

    sigma      = softplus(ro)
    sigma_bias = softplus(ro_bias)
    out[b,o]   = sum_i x[b,i] * (eps[b,i,o]*sigma[i,o] + mu[i,o])
                 + eps_bias[b,o]*sigma_bias[o] + mu_bias[o]

Sharding: pure data-parallel over the batch dim across 8 NeuronCores
(16 samples per core); mu/ro/mu_bias/ro_bias replicated per core.

Per-core kernel (memory-bound; eps dominates at 64MB/core):
  - sigma = Ln(exp(ro)+1) computed once into SBUF in the blocked layout
    [128, 8*1024] (partition p holds rows {blk*128+p : blk in 0..8}).
  - per sample: one 4MB DMA of eps[s] into the same blocked layout,
    one in-place DVE multiply by sigma, then 16 fp32r matmuls
    (lhsT = x-column [128,1], rhs = (eps*sigma) block halves [128,512])
    accumulating sum_i x[b,i]*A[i,o] into a [1,1024] PSUM tile.
  - x@mu handled once via 16 matmuls with lhsT = xT chunks [128,16].
  - bias terms and the x@mu result are combined and added in one final
    [16,1024] vector op before a single output DMA.
"""

from contextlib import ExitStack

import numpy as np

import concourse.bacc as bacc
import concourse.bass as bass
import concourse.mybir as mybir
import concourse.tile as tile
from concourse.bass_utils import run_bass_kernel_spmd

N_CORES = 8
B, IN, OUT = 128, 1024, 1024
B_L = B // N_CORES  # 16 samples per core
NBLK = IN // 128  # 8 row-blocks of 128 partitions
NH = OUT // 512  # 2 matmul halves (fp32 moving max is 512)
FREE = NBLK * OUT  # 8192 free elements per partition in blocked layout

_f32 = mybir.dt.float32
_f32r = mybir.dt.float32r
_AF = mybir.ActivationFunctionType

_NC_CACHE: dict = {}


def _build_nc(mm_dt=_f32r) -> bass.Bass:
    """Build the single-core Bass program (identical on all 8 cores)."""
    _f32r = mm_dt  # matmul operand dtype: float32r (fast) or float32 (precise)
    nc = bacc.Bacc("TRN2", target_bir_lowering=False, debug=False)

    eps = nc.dram_tensor("eps", [B_L, IN, OUT], _f32, kind="ExternalInput").ap()
    xt = nc.dram_tensor("xt", [128, NBLK * B_L], _f32, kind="ExternalInput").ap()
    ro = nc.dram_tensor("ro", [IN, OUT], _f32, kind="ExternalInput").ap()
    mu = nc.dram_tensor("mu", [IN, OUT], _f32, kind="ExternalInput").ap()
    eps_bias = nc.dram_tensor("eps_bias", [B_L, OUT], _f32, kind="ExternalInput").ap()
    ro_bias = nc.dram_tensor("ro_bias", [1, OUT], _f32, kind="ExternalInput").ap()
    mu_bias = nc.dram_tensor("mu_bias", [1, OUT], _f32, kind="ExternalInput").ap()
    out = nc.dram_tensor("out", [B_L, OUT], _f32, kind="ExternalOutput").ap()

    with tile.TileContext(nc) as tc, ExitStack() as ctx:
        consts = ctx.enter_context(tc.tile_pool(name="consts", bufs=1))
        mu_pool = ctx.enter_context(tc.tile_pool(name="mu", bufs=2))
        eps_pool = ctx.enter_context(tc.tile_pool(name="eps", bufs=3))
        psum2_pool = ctx.enter_context(tc.tile_pool(name="psum2", bufs=1, space="PSUM"))
        psum_eps_pool = ctx.enter_context(
            tc.tile_pool(name="psum_eps", bufs=3, space="PSUM")
        )
        small = ctx.enter_context(tc.tile_pool(name="small", bufs=1))
        rows = ctx.enter_context(tc.tile_pool(name="rows", bufs=4))

        # sigma = softplus(ro) in blocked layout, computed in place.
        SG = consts.tile([128, FREE], _f32)
        nc.sync.dma_start(
            out=SG[:].rearrange("p (blk o) -> p blk o", o=OUT),
            in_=ro.rearrange("(blk p) o -> p blk o", p=128),
        )
        nc.scalar.activation(SG[:], SG[:], _AF.Exp)
        nc.scalar.activation(SG[:], SG[:], _AF.Ln, bias=1.0)

        # xT chunks: XT[p, blk*B_L + b] = x[b, blk*128 + p]
        # (f32r output: the BIR verifier requires fp32r-matmul operands to be
        # produced as fp32r-rounded values)
        XT = consts.tile([128, NBLK * B_L], _f32)
        nc.sync.dma_start(out=XT[:].bitcast(_f32r), in_=xt.bitcast(_f32r))

        # bias = eps_bias * softplus(ro_bias) + mu_bias, on B_L partitions
        sgb = small.tile([B_L, OUT], _f32)
        nc.sync.dma_start(out=sgb[:], in_=ro_bias.broadcast_to((B_L, OUT)))
        nc.scalar.activation(sgb[:], sgb[:], _AF.Exp)
        nc.scalar.activation(sgb[:], sgb[:], _AF.Ln, bias=1.0)
        mub = small.tile([B_L, OUT], _f32)
        nc.sync.dma_start(out=mub[:], in_=mu_bias.broadcast_to((B_L, OUT)))
        eb = small.tile([B_L, OUT], _f32)
        nc.sync.dma_start(out=eb[:], in_=eps_bias[:])
        bias = small.tile([B_L, OUT], _f32)
        nc.vector.tensor_mul(bias[:], eb[:], sgb[:])
        nc.vector.tensor_add(bias[:], bias[:], mub[:])

        # out2 = x @ mu, accumulated over the 8 row-blocks
        psum2 = psum2_pool.tile([B_L, OUT], _f32)
        mu_r = mu.rearrange("(blk p) o -> p blk o", p=128)
        for blk in range(NBLK):
            mu_t = mu_pool.tile([128, OUT], _f32)
            nc.sync.dma_start(out=mu_t[:].bitcast(_f32r), in_=mu_r[:, blk, :].bitcast(_f32r))
            for h in range(NH):
                nc.tensor.matmul(
                    psum2[:, h * 512 : (h + 1) * 512],
                    lhsT=XT[:, blk * B_L : (blk + 1) * B_L].bitcast(_f32r),
                    rhs=mu_t[:, h * 512 : (h + 1) * 512].bitcast(_f32r),
                    start=(blk == 0),
                    stop=(blk == NBLK - 1),
                )
        O2B = small.tile([B_L, OUT], _f32)
        nc.vector.tensor_add(O2B[:], psum2[:], bias[:])

        # main loop: one eps mega-tile per sample
        stage = small.tile([B_L, OUT], _f32)
        for s in range(B_L):
            E = eps_pool.tile([128, FREE], _f32)
            nc.sync.dma_start(
                out=E[:].rearrange("p (blk o) -> p blk o", o=OUT).bitcast(_f32r),
                in_=eps[s].rearrange("(blk p) o -> p blk o", p=128).bitcast(_f32r),
            )
            # A = eps*sigma in place; f32r output dtype for the fp32r matmuls
            nc.vector.tensor_mul(E[:].bitcast(_f32r), E[:], SG[:])
            pse = psum_eps_pool.tile([1, OUT], _f32)
            for blk in range(NBLK):
                for h in range(NH):
                    nc.tensor.matmul(
                        pse[0:1, h * 512 : (h + 1) * 512],
                        lhsT=XT[:, blk * B_L + s : blk * B_L + s + 1].bitcast(_f32r),
                        rhs=E[
                            :, blk * OUT + h * 512 : blk * OUT + h * 512 + 512
                        ].bitcast(_f32r),
                        start=(blk == 0),
                        stop=(blk == NBLK - 1),
                    )
            # PSUM row -> SBUF row (partition 0), then SBUF->SBUF DMA to row s.
            # On DVE (not ACT) so the next sample's first matmul sees its data
            # dep (E mult) and its PSUM-recycle dep on the same semaphore.
            srow = rows.tile([1, OUT], _f32)
            nc.vector.tensor_copy(srow[:], pse[:])
            nc.sync.dma_start(out=stage[s : s + 1, :], in_=srow[0:1, :])

        out_sb = small.tile([B_L, OUT], _f32)
        nc.vector.tensor_add(out_sb[:], stage[:], O2B[:])
        nc.sync.dma_start(out=out[:], in_=out_sb[:])

    nc.compile()
    return nc


def _get_nc() -> bass.Bass:
    if "nc" not in _NC_CACHE:
        _NC_CACHE["nc"] = _build_nc()
    return _NC_CACHE["nc"]


def _make_in_maps(x, eps, eps_bias, mu, ro, mu_bias, ro_bias):
    x = np.asarray(x, dtype=np.float32)
    eps = np.asarray(eps, dtype=np.float32)
    eps_bias = np.asarray(eps_bias, dtype=np.float32)
    mu = np.ascontiguousarray(np.asarray(mu, dtype=np.float32))
    ro = np.ascontiguousarray(np.asarray(ro, dtype=np.float32))
    mu_bias = np.ascontiguousarray(np.asarray(mu_bias, dtype=np.float32))
    ro_bias = np.ascontiguousarray(np.asarray(ro_bias, dtype=np.float32))

    in_maps = []
    for c in range(N_CORES):
        sl = slice(c * B_L, (c + 1) * B_L)
        x_l = x[sl]  # [B_L, IN]
        # xt[p, blk*B_L + b] = x_l[b, blk*128 + p]
        xt = np.ascontiguousarray(
            x_l.T.reshape(NBLK, 128, B_L).transpose(1, 0, 2).reshape(128, NBLK * B_L)
        )
        in_maps.append(
            {
                "eps": np.ascontiguousarray(eps[sl]),
                "xt": xt,
                "ro": ro,
                "mu": mu,
                "eps_bias": np.ascontiguousarray(eps_bias[sl]),
                "ro_bias": ro_bias,
                "mu_bias": mu_bias,
            }
        )
    return in_maps


def _run(in_maps, trace=False, **kwargs):
    nc = _get_nc()
    return run_bass_kernel_spmd(
        nc, in_maps, core_ids=list(range(N_CORES)), trace=trace, **kwargs
    )


def kernel(x, eps, eps_bias, mu, ro, mu_bias, ro_bias):
    in_maps = _make_in_maps(x, eps, eps_bias, mu, ro, mu_bias, ro_bias)
    res = _run(in_maps, trace=False)
    return np.concatenate(
        [res.results[c]["out"] for c in range(N_CORES)], axis=0
    ).astype(np.float32)
